# revision 20
# baseline (speedup 1.0000x reference)
"""CSWin attention Trainium2 kernel (v2).

Shapes (hardcoded): B=8, H=W=64, N=4096, C=512, 8 heads (4 horizontal-stripe,
4 vertical-stripe), head_dim=64, stripe width SPLIT=8.

Sharding: data-parallel over batch B across the 8 NeuronCores (1 image/core).

Host staging (in kernel()): x is cast to bf16 and pre-transposed to xT
[C, N]; Wqkv/Wproj are pre-transposed+cast; the LePE diagonal tiles, the
identity, and all bias layouts are prebuilt on host.  This removes the
entire on-device transpose/cast preamble.

On-chip (per core, matmuls bf16 with fp32 PSUM):
  - qkvT [1536, 4096] = WqkvT @ xT, bias fused into the PSUM->SBUF copy
    (alternating DVE tensor_scalar_add / ScalarE Identity+bias).  v-half
    head channels written column-major so vertical stripes are contiguous.
  - attention: two interleaved streams of head-pairs (v-half stripes first,
    then h-half), software-pipelined one pair ahead:
      * LePE: 9 K=128 diagonal matmuls with shifted windows accumulating
        v + conv(v) in PSUM (lazy-zero sub-blocks, single group);
      * scoresT [k, q] per head, exp on ScalarE out of PSUM;
      * AV flipped: out[q, d] = sum_k E[k,q] v_lepe[k,d] with a ones
        column in the rhs so col 64 is the softmax denominator; 16
        matmuls of N=65 per head accumulate into one [128, 260] PSUM
        tile (4 q-blocks, lazy-zero);
      * normalization: DVE reciprocal [128,4] + per-partition-scalar
        multiplies into a token-major pair slab, PE transpose back to
        channel-major, copy into concatT.
  - proj: token-major PSUM matmuls from concatT + WprojT, bias added via
    DVE tensor_add with a host-broadcast bias tile, DMA out.
"""

import numpy as np

import concourse.bass as bass
import concourse.bacc as bacc
import concourse.mybir as mybir
from concourse import bass_utils
from concourse.tile import TileContext

F32 = mybir.dt.float32
F16 = mybir.dt.float16
BF16 = mybir.dt.bfloat16
FP8 = mybir.dt.float8e4
WSCALE = 64.0

B = 8
H = 64
W = 64
N = H * W          # 4096
C = 512
NH = 8             # heads
HD = 64            # head dim
SP = 8             # stripe width
NS = 8             # stripes per direction
SCALE = HD ** -0.5

_CACHE = {}


def _build_nc():
    nc = bacc.Bacc("TRN2", target_bir_lowering=False, debug=False)

    # fp8 hi/lo split of x / Wqkv in DoubleRow pair-interleaved layout:
    # x planes [128, 8192]: col nt*1024 + c2*512 + (t%512), channel pair
    # (cc, cc+1) on c2; w planes [128, 3072]: col jt*256 + c2*128 + j.
    x8_d = [nc.dram_tensor(f"x8{ab}", (128, 2 * N), FP8,
                           kind="ExternalInput").ap() for ab in "ab"]
    xr_d = [nc.dram_tensor(f"xr{ab}", (128, 2 * N), FP8,
                           kind="ExternalInput").ap() for ab in "ab"]
    w8_d = [nc.dram_tensor(f"w8{ab}", (128, 6 * C), FP8,
                           kind="ExternalInput").ap() for ab in "ab"]
    wr_d = [nc.dram_tensor(f"wr{ab}", (128, 6 * C), FP8,
                           kind="ExternalInput").ap() for ab in "ab"]
    wprojT_d = nc.dram_tensor("wprojT", (C, C), BF16, kind="ExternalInput").ap()
    dg_d = nc.dram_tensor("dg", (128, 19 * 128), BF16, kind="ExternalInput").ap()
    bqkv_d = nc.dram_tensor("bqkv12", (128, 12), F32, kind="ExternalInput").ap()
    lepeb_d = nc.dram_tensor("lepeb", (128, 2), F32, kind="ExternalInput").ap()
    bprojb_d = nc.dram_tensor("bprojb", (128, C), F32, kind="ExternalInput").ap()
    y_d = nc.dram_tensor("y", (N, C), F16, kind="ExternalOutput").ap()

    with TileContext(nc) as tc:
        _emit(nc, tc, x8_d, xr_d, w8_d, wr_d, wprojT_d, dg_d, bqkv_d,
              lepeb_d, bprojb_d, y_d)
    nc.compile()
    return nc


def _emit(nc, tc, x8_d, xr_d, w8_d, wr_d, wprojT_d, dg_d, bqkv_d,
          lepeb_d, bprojb_d, y_d):
    import contextlib
    ctx = contextlib.ExitStack()
    with ctx:
        persist = ctx.enter_context(tc.tile_pool(name="persist", bufs=1))
        qkv_pool = ctx.enter_context(tc.tile_pool(name="qkvT", bufs=1))

        # ---------------- constants / weights (host-staged) ----------------
        wq8 = [persist.tile([128, 6 * C], FP8, name=f"wq8{ab}", tag=f"wq8{ab}")
               for ab in "ab"]
        wqr = [persist.tile([128, 6 * C], FP8, name=f"wqr{ab}", tag=f"wqr{ab}")
               for ab in "ab"]
        bqkv_sb = persist.tile([128, 12], F32, tag="bqkv")
        wprojT = [persist.tile([128, C], BF16, name=f"wprojT{fc}", tag=f"wprojT{fc}")
                  for fc in range(4)]
        dg = persist.tile([128, 19 * 128], BF16, tag="dg")
        id128 = dg[:, 18 * 128:19 * 128]

        def diag(half, k):
            return dg[:, (half * 9 + k) * 128:(half * 9 + k + 1) * 128]

        lepeb = persist.tile([128, 2], F32, tag="lepeb")
        bprojb = persist.tile([128, C], F32, tag="bprojb")

        def dma_weights_tail(step):
            if step < 2:
                g = step + 1
                for wt, wd in ((wq8, w8_d), (wqr, wr_d)):
                    for h in range(2):
                        nc.sync.dma_start(
                            out=wt[h][:, g * 1024:(g + 1) * 1024],
                            in_=wd[h][:, g * 1024:(g + 1) * 1024])
                if step == 1:
                    nc.sync.dma_start(out=lepeb, in_=lepeb_d)
            elif step == 2:
                nc.sync.dma_start(out=dg, in_=dg_d)
            else:
                for fc in range(4):
                    nc.sync.dma_start(out=wprojT[fc],
                                      in_=wprojT_d[fc * 128:(fc + 1) * 128, :])
                nc.sync.dma_start(out=bprojb, in_=bprojb_d)

        # ---------------- P1: qkvT [1536, 4096] ----------------
        # qkvT tile jt holds channels [128*jt, 128*(jt+1)): jt 0-3 q, 4-7 k,
        # 8-11 v; within a group tiles 0-1 = h-half heads (row-major
        # tokens), 2-3 = v-half (column-major token order t' = x*64 + y).
        qkvT = [qkv_pool.tile([128, N], BF16, name=f"qkvT{jt}", tag=f"qkvT{jt}")
                for jt in range(12)]
        with tc.tile_pool(name="xT", bufs=1) as xT_pool:
            x8 = [xT_pool.tile([128, 2 * N], FP8, name=f"x8{ab}", tag=f"x8{ab}")
                  for ab in "ab"]
            xr = [xT_pool.tile([128, 2 * N], FP8, name=f"xr{ab}", tag=f"xr{ab}")
                  for ab in "ab"]
            # critical-path first: w group-0 cols, then token chunks of the
            # fp8 x planes; remaining weights spread across later chunks
            for wt, wd in ((wq8, w8_d), (wqr, wr_d)):
                for h in range(2):
                    nc.sync.dma_start(out=wt[h][:, 0:1024],
                                      in_=wd[h][:, 0:1024])
            nc.sync.dma_start(out=bqkv_sb, in_=bqkv_d)
            for hf in range(4):
                for xt, xd in ((x8, x8_d), (xr, xr_d)):
                    for h in range(2):
                        nc.sync.dma_start(
                            out=xt[h][:, hf * 2048:(hf + 1) * 2048],
                            in_=xd[h][:, hf * 2048:(hf + 1) * 2048])
                dma_weights_tail(hf)

            with tc.tile_pool(name="qkv_psum", bufs=8, space="PSUM") as qkv_psum:
                DR = mybir.MatmulPerfMode.DoubleRow

                def drap(t, col0, inner, pair_stride):
                    return bass.AP(tensor=t.tensor, offset=t.offset + col0,
                                   ap=[t.ap[0], [pair_stride, 2], [1, inner]])

                # nt-outer so compute starts after the first x chunk;
                # v-half-related jts first so attention can start early
                jts = [2, 6, 10, 3, 7, 11, 0, 4, 8, 1, 5, 9]
                terms = [(0, 0), (1, 1), (0, 2), (1, 3), (2, 0), (3, 1)]
                # (w-idx in [wq8a,wq8b,wra,wrb], x-idx in [x8a,x8b,xra,xrb])
                for nt in range(8):
                    for ji, jt in enumerate(jts):
                        vhalf = (jt % 4) >= 2
                        ps = qkv_psum.tile([128, 512], F32, tag="qkvps")
                        for ti, (wi, xi) in enumerate(terms):
                            wt = (wq8 + wqr)[wi]
                            xt = (x8 + xr)[xi]
                            nc.tensor.matmul(
                                ps,
                                drap(wt, jt * 256, 128, 128),
                                drap(xt, nt * 1024, 512, 512),
                                start=(ti == 0), stop=(ti == 5),
                                perf_mode=DR)
                        if vhalf:
                            # scatter token chunk (rows y in [8nt, 8nt+8),
                            # all x) into column-major: addr = x*64 + y
                            out_ap = bass.AP(
                                tensor=qkvT[jt].tensor,
                                offset=qkvT[jt].offset + 8 * nt,
                                ap=[qkvT[jt].ap[0], [1, 8], [64, 64]])
                        else:
                            out_ap = qkvT[jt][:, nt * 512:(nt + 1) * 512]
                        if (ji + nt) % 2 == 0:
                            nc.vector.tensor_scalar(
                                out_ap, ps, 1.0 / WSCALE,
                                bqkv_sb[:, jt:jt + 1],
                                mybir.AluOpType.mult, mybir.AluOpType.add)
                        else:
                            nc.scalar.activation(
                                out_ap, ps,
                                mybir.ActivationFunctionType.Identity,
                                bias=bqkv_sb[:, jt:jt + 1], scale=1.0 / WSCALE)

        # ---------------- P2: attention ----------------
        concatT = [persist.tile([128, N], BF16, name=f"concatT{fc}", tag=f"concatT{fc}")
                   for fc in range(4)]

        # stream st handles hp=st; v-half stripes first, then h-half
        streams = [[(1, s, st) for s in range(NS)] + [(0, s, st) for s in range(NS)]
                   for st in (0, 1)]

        # PSUM (8 banks): sps/pt shared tag 2x[128,1024]f32 (4) +
        # lp 2x[128,512] (vl/vnp/proj-ps, 2) + oa 2x[128,260]f32 (2)
        with tc.tile_pool(name="sc_psum", bufs=2, space="PSUM") as sc_psum, \
             tc.tile_pool(name="lp_psum", bufs=1, space="PSUM") as lp_psum, \
             tc.tile_pool(name="oa_psum", bufs=2, space="PSUM") as oa_psum, \
             tc.tile_pool(name="pt_psum", bufs=1, space="PSUM") as pt_psum, \
             tc.tile_pool(name="att", bufs=6) as att, \
             tc.tile_pool(name="lepe_sb", bufs=3) as lepe_sb, \
             tc.tile_pool(name="vna_sb", bufs=3) as vna_sb, \
             tc.tile_pool(name="po_sb", bufs=3) as po_sb, \
             tc.tile_pool(name="rec_sb", bufs=4) as rec_sb, \
             tc.tile_pool(name="pj_sb", bufs=4) as pj_sb:

            def emit_lepe(pr):
                """LePE for pair pr -> vlsb SBUF pair slab [128, 512]."""
                half, s, hp = pr
                tok0 = s * 512
                vt = qkvT[8 + half * 2 + hp]
                vsp = vt[:, tok0:tok0 + 512]
                vl = lp_psum.tile([128, 512], F32, tag="lps", name="vl")
                order = [4, 0, 1, 2, 3, 5, 6, 7, 8]
                for ki, k in enumerate(order):
                    dr, dc = k // 3 - 1, k % 3 - 1
                    r0 = max(0, -dr)
                    nr = 8 - abs(dr)
                    x0 = max(0, -dc)
                    nx = 64 - abs(dc)
                    o_off = r0 * 64 + x0
                    i_off = (r0 + dr) * 64 + (x0 + dc)
                    if dc == 0:
                        out_ap = vl[:, o_off:o_off + nr * 64]
                        in_ap = vsp[:, i_off:i_off + nr * 64]
                    else:
                        out_ap = bass.AP(
                            tensor=vl.tensor, offset=vl.offset + o_off,
                            ap=[vl.ap[0], [64, nr], [1, nx]])
                        in_ap = bass.AP(
                            tensor=vsp.tensor, offset=vsp.offset + i_off,
                            ap=[vsp.ap[0], [64, nr], [1, nx]])
                    nc.tensor.matmul(
                        out_ap, diag(half, k), in_ap,
                        start=(ki == 0), stop=(ki == len(order) - 1),
                        skip_group_check=True)
                return (vl, half)

            def emit_vna(pr, vlh):
                """vl PSUM -> vlsb (DVE, +bias), transpose, build pair
                v_nat_aug [128, 520]: chunk (jc, head) at cols
                (jc*2+head)*65, col 64 = ones."""
                vl, half = vlh
                vlsb = lepe_sb.tile([128, 512], BF16, tag="vlsb", name="vlsb")
                nc.scalar.activation(
                    vlsb, vl, mybir.ActivationFunctionType.Identity,
                    bias=lepeb[:, half:half + 1], scale=1.0)
                vnp = lp_psum.tile([128, 512], BF16, tag="lps", name="vnp")
                for jc in range(4):
                    nc.tensor.transpose(
                        vnp[:, jc * 128:(jc + 1) * 128],
                        vlsb[:, jc * 128:(jc + 1) * 128], id128)
                vna = vna_sb.tile([128, 520], BF16, tag="vna", name="vna")
                vna_data = bass.AP(
                    tensor=vna.tensor, offset=vna.offset,
                    ap=[vna.ap[0], [130, 4], [65, 2], [1, 64]])
                nc.vector.tensor_copy(vna_data, vnp)
                vna_ones = bass.AP(
                    tensor=vna.tensor, offset=vna.offset + 64,
                    ap=[vna.ap[0], [130, 4], [65, 2]])
                nc.gpsimd.memset(vna_ones, 1.0)
                return vna

            def emit_pair_scores(pr):
                """scoresT + exp for BOTH heads of pair pr (chunk-
                interleaved on disjoint PE row groups); returns esb x2."""
                half, s, hp = pr
                tok0 = s * 512
                jt_off = half * 2 + hp
                esbs = []
                qkss = []
                for hh in range(2):
                    pbase = hh * 64
                    qkss.append((
                        qkvT[jt_off][pbase:pbase + 64, tok0:tok0 + 512],
                        qkvT[4 + jt_off][pbase:pbase + 64, tok0:tok0 + 512]))
                    esbs.append(att.tile([128, 2048], BF16, tag="esb",
                                         name="esb"))
                for hh in range(2):
                    for sh in range(2):
                        qs, ks = qkss[hh]
                        sps = sc_psum.tile([128, 1024], F32, tag="sps",
                                           name="sps")
                        for jj in range(2):
                            jc = 2 * sh + jj
                            nc.tensor.matmul(
                                sps[:, jj * 512:(jj + 1) * 512],
                                ks[:, jc * 128:(jc + 1) * 128], qs,
                                start=True, stop=True)
                        # exp; no max subtraction needed (scores ~ N(0,1))
                        nc.scalar.activation(
                            esbs[hh][:, sh * 1024:(sh + 1) * 1024], sps,
                            mybir.ActivationFunctionType.Exp,
                            bias=0.0, scale=SCALE)
                return esbs

            def emit_avh(pr, hh, vna, esb, po, par):
                """AV (flipped) for head hh of pair pr: out[q, d] in oa
                [128, 4*65], then reciprocal + scale into po slab."""
                half, s, hp = pr
                oa = oa_psum.tile([128, 260], F32, tag="oa", name="oa")
                for qb in range(4):
                    for jc in range(4):
                        nc.tensor.matmul(
                            oa[:, qb * 65:qb * 65 + 65],
                            esb[:, jc * 512 + qb * 128:jc * 512 + qb * 128 + 128],
                            vna[:, (jc * 2 + hh) * 65:(jc * 2 + hh) * 65 + 65],
                            start=(qb == 0 and jc == 0),
                            stop=(qb == 3 and jc == 3),
                            skip_group_check=True)
                rec = rec_sb.tile([128, 4], F32, tag="rec", name="rec")
                den_ap = bass.AP(tensor=oa.tensor, offset=oa.offset + 64,
                                 ap=[oa.ap[0], [65, 4]])
                nc.vector.reciprocal(rec, den_ap)
                dst = bass.AP(
                    tensor=po.tensor, offset=po.offset + hh * 64,
                    ap=[po.ap[0], [128, 4], [1, 64]])
                srcap = bass.AP(
                    tensor=oa.tensor, offset=oa.offset,
                    ap=[oa.ap[0], [65, 4], [1, 64]])
                recb = bass.AP(
                    tensor=rec.tensor, offset=rec.offset,
                    ap=[rec.ap[0], [1, 4], [0, 64]])
                nc.vector.tensor_mul(dst, srcap, recb)

            po_q = []        # deferred per-qb poout closures
            proj_ready = []  # (tt, slot when unlocked)
            h_done = {}      # (s, qb) -> # of h-half pieces drained
            slot_no = [0]

            def poout_piece(pr, po, qb):
                half, s, hp = pr
                fc = (2 if half else 0) + hp
                cfc = concatT[fc]
                pt = pt_psum.tile([128, 128], BF16, tag="pt", name="pt")
                nc.tensor.transpose(pt, po[:, qb * 128:(qb + 1) * 128],
                                    id128)
                if half == 0:
                    out_ap = cfc[:, s * 512 + qb * 128:s * 512 + (qb + 1) * 128]
                    in_ap = pt
                else:
                    # local col j = xi*64 + y -> t = y*64 + 8s + 2qb + xi
                    out_ap = bass.AP(
                        tensor=cfc.tensor,
                        offset=cfc.offset + 8 * s + 2 * qb,
                        ap=[cfc.ap[0], [1, 2], [64, 64]])
                    in_ap = bass.AP(
                        tensor=pt.tensor, offset=pt.offset,
                        ap=[pt.ap[0], [64, 2], [1, 64]])
                nc.vector.tensor_copy(out_ap, in_ap)
                if half == 0:
                    n = h_done.get((s, qb), 0) + 1
                    h_done[(s, qb)] = n


            def proj_tt(tt):
                ps = oa_psum.tile([128, C], F32, tag="oa", name="pjps")
                for fc in range(4):
                    nc.tensor.matmul(
                        ps, concatT[fc][:, tt * 128:(tt + 1) * 128],
                        wprojT[fc],
                        start=(fc == 0), stop=(fc == 3))
                osb = pj_sb.tile([128, C], F16, tag="pjout", name="osb")
                nc.vector.tensor_add(osb, ps, bprojb)
                nc.sync.dma_start(
                    out=y_d[tt * 128:(tt + 1) * 128, :], in_=osb)

            def emit_poout(pr, po, par):
                """Queue the 4 transpose+copy pieces; drain() weaves them
                (and any unlocked proj blocks) between other PE work."""
                for qb in range(4):
                    po_q.append((pr, po, qb))

            def drain(k=1, flush=False):
                for _ in range(k):
                    if po_q:
                        poout_piece(*po_q.pop(0))
                    else:
                        break

            def emit_proj(s):
                for tt in range(4 * s, 4 * s + 4):
                    proj_tt(tt)

            # two streams interleaved; LePE/vna for pair i+1 pipelined
            # between the AV work of pair i; poout for stream1 deferred to
            # the start of the next step so its DVE chain has drained.
            vna_cur = []
            for st in (0, 1):
                vlh0 = emit_lepe(streams[st][0])
                vna_cur.append(emit_vna(streams[st][0], vlh0))
            nsteps = len(streams[0])
            for i in range(nsteps):
                p = [streams[0][i], streams[1][i]]
                nxt = [streams[st][i + 1] if i + 1 < nsteps else None
                       for st in (0, 1)]
                esbA = emit_pair_scores(p[0])
                drain(1)
                vlsb_next = [None, None]
                if nxt[0] is not None:
                    vlsb_next[0] = emit_lepe(nxt[0])
                if i >= 9:
                    # h-step i-1 handled h-stripe s=i-9; all its poout
                    # pieces must land before proj reads concatT
                    drain(len(po_q))
                    emit_proj(i - 9)
                drain(1)
                poA = po_sb.tile([128, 512], BF16, tag="po", name="po")
                emit_avh(p[0], 0, vna_cur[0], esbA[0], poA, i)
                drain(2)
                esbB = emit_pair_scores(p[1])
                drain(1)
                emit_avh(p[0], 1, vna_cur[0], esbA[1], poA, i)
                if nxt[1] is not None:
                    vlsb_next[1] = emit_lepe(nxt[1])
                emit_poout(p[0], poA, i)
                drain(1)
                poB = po_sb.tile([128, 512], BF16, tag="po", name="po")
                emit_avh(p[1], 0, vna_cur[1], esbB[0], poB, i + 1)
                drain(1)
                if nxt[0] is not None:
                    vna_cur[0] = emit_vna(nxt[0], vlsb_next[0])
                drain(1)
                emit_avh(p[1], 1, vna_cur[1], esbB[1], poB, i + 1)
                if nxt[1] is not None:
                    vna_cur[1] = emit_vna(nxt[1], vlsb_next[1])
                emit_poout(p[1], poB, i + 1)
                drain(2)
            # final stripe: weave proj blocks between the piece drains
            done7 = set()
            while po_q or len(done7) < 4:
                drain(2)
                for qb in range(4):
                    if qb not in done7 and h_done.get((7, qb), 0) == 2:
                        proj_tt(28 + qb)
                        done7.add(qb)


def _get_nc():
    if "nc" not in _CACHE:
        _CACHE["nc"] = _build_nc()
    return _CACHE["nc"]


def _host_stage(inputs):
    import ml_dtypes
    bf16 = ml_dtypes.bfloat16
    wqkv = np.asarray(inputs["Wqkv"], np.float32)
    wproj = np.asarray(inputs["Wproj"], np.float32)
    bqkv = np.asarray(inputs["bqkv"], np.float32)
    bproj = np.asarray(inputs["bproj"], np.float32)
    lhw = np.asarray(inputs["lepe_h_w"], np.float32).reshape(9, HD)
    lvw = np.asarray(inputs["lepe_v_w"], np.float32).reshape(9, HD)
    lhb = np.asarray(inputs["lepe_h_b"], np.float32)
    lvb = np.asarray(inputs["lepe_v_b"], np.float32)

    import concourse.mybir as _mb
    f8 = _mb.dt.np(_mb.dt.float8e4)
    # Wqkv scaled by WSCALE, split hi/lo fp8, DoubleRow pair layout:
    # w{8,r}{a,b} [128, 3072]: col jt*256 + c2*128 + j  holds
    # W[jt*128+j, base + c2*128 + p] for partition p (contraction channel).
    ws = wqkv.T * WSCALE                                       # [C, 3C]
    w8f = ws.astype(f8).astype(np.float32)
    wrf = (ws - w8f).astype(f8)
    w8f = w8f.astype(f8)

    def pair_w(a):                                             # [C, 3C] -> 2x[128, 3072]
        outs = []
        for half in range(2):
            blk = a[half * 256:(half + 1) * 256]               # [256, 3C]
            b4 = blk.reshape(2, 128, 12, 128)                  # c2, p, jt, j
            outs.append(np.ascontiguousarray(
                b4.transpose(1, 2, 0, 3).reshape(128, 12 * 256)))
        return outs

    w8a, w8b = pair_w(w8f)
    wra, wrb = pair_w(wrf)
    wprojT = np.ascontiguousarray(wproj.T).astype(bf16)        # [C, C]
    bqkv12 = np.ascontiguousarray(bqkv.reshape(12, 128).T)     # [128, 12]
    bprojb = np.ascontiguousarray(
        np.broadcast_to(bproj, (128, C))).astype(np.float32)   # [128, C]
    lepeb = np.stack([np.tile(lhb, 2), np.tile(lvb, 2)], axis=1)
    lepeb = np.ascontiguousarray(lepeb).astype(np.float32)     # [128, 2]

    # diag tiles [128, 19*128]: (half, k) at col (half*9+k)*128; block 18
    # is the identity.  Center tap (k=4) has I added (v + conv(v)).
    dg = np.zeros((128, 19 * 128), np.float32)
    for half, w9 in ((0, lhw), (1, lvw)):
        for k in range(9):
            dr, dc = k // 3 - 1, k % 3 - 1
            if half == 0:
                wi = (dr + 1) * 3 + (dc + 1)
            else:
                wi = (dc + 1) * 3 + (dr + 1)
            vals = np.tile(w9[wi], 2)                          # [128]
            d = np.diag(vals)
            if k == 4:
                d = d + np.eye(128, dtype=np.float32)
            dg[:, (half * 9 + k) * 128:(half * 9 + k + 1) * 128] = d
    dg[:, 18 * 128:19 * 128] = np.eye(128, dtype=np.float32)
    dg = dg.astype(bf16)

    return {
        "w8a": w8a, "w8b": w8b, "wra": wra, "wrb": wrb,
        "wprojT": wprojT, "dg": np.ascontiguousarray(dg),
        "bqkv12": bqkv12.astype(np.float32), "lepeb": lepeb,
        "bprojb": bprojb,
    }


def _pair_x(a):
    # [C, N] -> 2 x [128, 8192]: col nt*1024 + c2*512 + t%512
    outs = []
    for half in range(2):
        blk = a[half * 256:(half + 1) * 256]                   # [256, N]
        b4 = blk.reshape(2, 128, 8, 512)                       # c2, p, nt, t
        outs.append(np.ascontiguousarray(
            b4.transpose(1, 2, 0, 3).reshape(128, 8 * 1024)))
    return outs


def kernel(**inputs):
    import concourse.mybir as _mb
    f8 = _mb.dt.np(_mb.dt.float8e4)
    x = np.asarray(inputs["x"], dtype=np.float32)
    shared = _host_stage(inputs)
    nc = _get_nc()
    in_maps = []
    for b in range(B):
        m = dict(shared)
        xT = x[b].T                                            # [C, N] f32
        x8f = xT.astype(f8).astype(np.float32)
        xrf = (xT - x8f).astype(f8)
        x8f = x8f.astype(f8)
        m["x8a"], m["x8b"] = _pair_x(x8f)
        m["xra"], m["xrb"] = _pair_x(xrf)
        in_maps.append(m)
    res = bass_utils.run_bass_kernel_spmd(nc, in_maps, core_ids=list(range(B)))
    out = np.stack([res.results[b]["y"] for b in range(B)], axis=0)
    return out.astype(np.float32)


if __name__ == "__main__":
    rng = np.random.default_rng(0)
    ins = {
        "x": rng.standard_normal((B, N, C), dtype=np.float32),
        "Wqkv": rng.standard_normal((3 * C, C), dtype=np.float32) * C ** -0.5,
        "bqkv": np.zeros(3 * C, np.float32),
        "Wproj": rng.standard_normal((C, C), dtype=np.float32) * C ** -0.5,
        "bproj": np.zeros(C, np.float32),
        "lepe_h_w": rng.standard_normal((3, 3, 1, HD), dtype=np.float32) / 3,
        "lepe_h_b": np.zeros(HD, np.float32),
        "lepe_v_w": rng.standard_normal((3, 3, 1, HD), dtype=np.float32) / 3,
        "lepe_v_b": np.zeros(HD, np.float32),
        "H": np.int64(H), "W": np.int64(W),
    }
    out = kernel(**ins)
    print(out.shape, out.dtype)
